# revision 4
# baseline (speedup 1.0000x reference)
"""Trainium2 Bass kernel for nn_NewAttentionMLP (dense MLP trunk + attention MLP).

Strategy:
  - Pure data parallel over 8 NeuronCores: batch 64 -> 8 per core (2048 tokens).
  - All LayerNorms algebraically folded into the matmul weights on the host:
        LN(x) @ W + b  =  r .* (x @ What) + c
    where What = diag(g) W - ones (.) (g@W)/F  (mean term folded in),
          c = b_ln @ W + b, and r = 1/sqrt(E[x^2] - mu^2 + eps) per token.
  - Activations live feature-major [feat(partition), token(free)] so per-token
    LN statistics are computed on the tensor engine with ones-matmuls
    (replicated across all 128 output partitions, ready for broadcasting).
  - All matmuls run in float32r (TF32-like, 1 cycle/row at N>=256).
"""

import sys
import time

sys.path.insert(0, "/opt/trn_rl_repo")

import numpy as np

# Problem constants (hardcoded per harness contract)
BS, LN, FS = 64, 256, 512
H, OUT, ATT, L = 1024, 128, 256, 8
NCORES = 8
BPC = BS // NCORES  # batches per core = 8
TOK = BPC * LN  # tokens per core = 2048
NT = 4  # token chunks per core
TCH = TOK // NT  # 512 tokens per chunk
EPS = 1e-5
P = 128

_CACHE = {}


# ---------------------------------------------------------------- host folds
def _fold(g, b_ln, W, b_lin, F):
    """LN(x)@W + b_lin -> r*(x@What) + c."""
    g = np.asarray(g, np.float64)
    b_ln = np.asarray(b_ln, np.float64)
    W = np.asarray(W, np.float64)
    b_lin = np.asarray(b_lin, np.float64)
    Wg = W * g[:, None]
    s = Wg.sum(axis=0)
    What = Wg - s[None, :] / F
    c = b_ln @ W + b_lin
    return What.astype(np.float32), c.astype(np.float32)


def _tile_w(What, kb, mcols):
    """[F, M] -> [mcols, 128, kb, 128] with w[m, p, b, j] = What[b*128+p, m*128+j]."""
    F, M = What.shape
    assert F == kb * 128 and M == mcols * 128
    return np.ascontiguousarray(
        What.reshape(kb, 128, mcols, 128).transpose(2, 1, 0, 3)
    )


def _tile_c(c, mcols):
    """[M] -> [128, mcols] with out[p, m] = c[m*128+p]."""
    return np.ascontiguousarray(c.reshape(mcols, 128).T)


def _prepare(inputs):
    inp = {k: np.asarray(v, np.float32) for k, v in inputs.items()}

    w0, c0 = _fold(inp["ln0_g"], inp["ln0_b"], inp["W0"], inp["b0"], FS)
    wres, cres = [], []
    for i in range(L):
        wi, ci = _fold(
            inp["res_ln_g"][i], inp["res_ln_b"][i], inp["res_W"][i], inp["res_b"][i], H
        )
        wres.append(wi)
        cres.append(ci)
    wf, cf = _fold(inp["lnf_g"], inp["lnf_b"], inp["Wf"], inp["bf"], H)
    wa1, ca1 = _fold(inp["a_ln1_g"], inp["a_ln1_b"], inp["Wa1"], inp["ba1"], LN + 1)
    wa2, ca2 = _fold(inp["a_ln2_g"], inp["a_ln2_b"], inp["Wa2"], inp["ba2"], ATT)

    consts = np.zeros((128, 6, 128), np.float32)
    consts[:, 0, :] = 1.0 / FS  # inv512
    consts[:, 1, :] = 1.0 / H  # inv1024
    consts[:, 2, :] = 1.0 / (LN + 1)  # inv257
    consts[:, 3, :] = 1.0 / ATT  # inv256
    consts[:, 4, :] = 1.0  # ones
    consts[:, 5, :] = np.eye(128, dtype=np.float32)  # identity

    shared = {
        "w0": _tile_w(w0, 4, 8),
        "wres": np.stack([_tile_w(w, 8, 8) for w in wres]),
        "wf": _tile_w(wf, 8, 1)[0],  # [128, 8, 128]
        "wa1": _tile_w(wa1[: 2 * 128], 2, 2),  # first 256 rows
        "wa1l": np.ascontiguousarray(wa1[2 * 128 :]),  # [1, 256]
        "wa2": _tile_w(wa2, 2, 2),
        "c0": _tile_c(c0, 8),
        "cres": np.ascontiguousarray(
            np.stack([_tile_c(c, 8) for c in cres]).transpose(1, 0, 2)
        ),  # [128, L, 8]
        "cf": _tile_c(cf, 1),
        "ca1": _tile_c(ca1, 2),
        "ca2": _tile_c(ca2, 2),
        "consts": consts,
        "eps": np.full((128, 1), EPS, np.float32),
    }

    x = inp["x"]  # [64, 256, 512]
    maps = []
    for c in range(NCORES):
        m = dict(shared)
        m["x"] = np.ascontiguousarray(x[c * BPC : (c + 1) * BPC].reshape(TOK, FS))
        maps.append(m)
    return maps


# ---------------------------------------------------------------- bass build
def _build():
    import concourse.mybir as mybir
    import concourse.tile as tile
    from concourse import bacc

    f32 = mybir.dt.float32
    f32r = mybir.dt.float32r
    AF = mybir.ActivationFunctionType

    nc = bacc.Bacc(None, target_bir_lowering=False)

    x_d = nc.dram_tensor("x", (TOK, FS), f32r, kind="ExternalInput")
    w0_d = nc.dram_tensor("w0", (8, 128, 4, 128), f32r, kind="ExternalInput")
    wres_d = nc.dram_tensor("wres", (L, 8, 128, 8, 128), f32r, kind="ExternalInput")
    wf_d = nc.dram_tensor("wf", (128, 8, 128), f32r, kind="ExternalInput")
    wa1_d = nc.dram_tensor("wa1", (2, 128, 2, 128), f32r, kind="ExternalInput")
    wa1l_d = nc.dram_tensor("wa1l", (1, 256), f32r, kind="ExternalInput")
    wa2_d = nc.dram_tensor("wa2", (2, 128, 2, 128), f32r, kind="ExternalInput")
    c0_d = nc.dram_tensor("c0", (128, 8), f32, kind="ExternalInput")
    cres_d = nc.dram_tensor("cres", (128, L, 8), f32, kind="ExternalInput")
    cf_d = nc.dram_tensor("cf", (128, 1), f32, kind="ExternalInput")
    ca1_d = nc.dram_tensor("ca1", (128, 2), f32, kind="ExternalInput")
    ca2_d = nc.dram_tensor("ca2", (128, 2), f32, kind="ExternalInput")
    consts_d = nc.dram_tensor("consts", (128, 6, 128), f32r, kind="ExternalInput")
    eps_d = nc.dram_tensor("eps", (128, 1), f32, kind="ExternalInput")
    out_d = nc.dram_tensor("out", (1, BPC * OUT), f32, kind="ExternalOutput")

    with tile.TileContext(nc) as tc:
        _emit(nc, tc, locals())
    nc.compile()
    return nc


def _emit(nc, tc, d):
    from contextlib import ExitStack

    import concourse.mybir as mybir

    f32 = mybir.dt.float32
    f32r = mybir.dt.float32r
    AF = mybir.ActivationFunctionType

    x_d, w0_d, wres_d, wf_d = d["x_d"], d["w0_d"], d["wres_d"], d["wf_d"]
    wa1_d, wa1l_d, wa2_d = d["wa1_d"], d["wa1l_d"], d["wa2_d"]
    c0_d, cres_d, cf_d, ca1_d, ca2_d = (
        d["c0_d"],
        d["cres_d"],
        d["cf_d"],
        d["ca1_d"],
        d["ca2_d"],
    )
    consts_d, eps_d, out_d = d["consts_d"], d["eps_d"], d["out_d"]

    with ExitStack() as ctx:
        pool_const = ctx.enter_context(tc.tile_pool(name="const", bufs=1))
        pool_h = ctx.enter_context(tc.tile_pool(name="h", bufs=32))
        pool_w = ctx.enter_context(tc.tile_pool(name="w", bufs=10))
        pool_sq = ctx.enter_context(tc.tile_pool(name="sq", bufs=4))
        pool_u = ctx.enter_context(tc.tile_pool(name="u", bufs=3))
        pool_tr = ctx.enter_context(tc.tile_pool(name="tr", bufs=4))
        pool_r = ctx.enter_context(tc.tile_pool(name="rr", bufs=5))
        pool_ps = ctx.enter_context(tc.tile_pool(name="ps", bufs=4, space="PSUM"))

        # ---- constants
        consts_sb = pool_const.tile([128, 6, 128], f32r, name="consts_sb")
        nc.sync.dma_start(out=consts_sb, in_=consts_d[:])
        inv512 = consts_sb[:, 0, :]
        inv1024 = consts_sb[:, 1, :]
        inv257 = consts_sb[:, 2, :]
        inv256 = consts_sb[:, 3, :]
        ones_c = consts_sb[:, 4, :]
        ident = consts_sb[:, 5, :]
        c0_sb = pool_const.tile([128, 8], f32, name="c0_sb")
        nc.sync.dma_start(out=c0_sb, in_=c0_d[:])
        cres_sb = pool_const.tile([128, L, 8], f32, name="cres_sb")
        nc.sync.dma_start(out=cres_sb, in_=cres_d[:])
        cf_sb = pool_const.tile([128, 1], f32, name="cf_sb")
        nc.sync.dma_start(out=cf_sb, in_=cf_d[:])
        ca1_sb = pool_const.tile([128, 2], f32, name="ca1_sb")
        nc.sync.dma_start(out=ca1_sb, in_=ca1_d[:])
        ca2_sb = pool_const.tile([128, 2], f32, name="ca2_sb")
        nc.sync.dma_start(out=ca2_sb, in_=ca2_d[:])
        eps_sb = pool_const.tile([128, 1], f32, name="eps_sb")
        nc.sync.dma_start(out=eps_sb, in_=eps_d[:])

        h_tiles = [[None] * NT for _ in range(8)]  # [m][T]
        r_tiles = [None] * NT

        def stat_r(S1, S2, T, name):
            """r = 1/sqrt(S2 - S1^2 + eps) on [128, TCH] replicated tiles."""
            m2 = pool_tr.tile([128, TCH], f32, tag="tr", name=f"m2_{name}")
            nc.scalar.activation(m2, S1, AF.Square)
            vv = pool_tr.tile([128, TCH], f32, tag="tr", name=f"vv_{name}")
            nc.vector.tensor_sub(vv, S2, m2)
            sd = pool_tr.tile([128, TCH], f32, tag="tr", name=f"sd_{name}")
            nc.scalar.activation(sd, vv, AF.Sqrt, bias=eps_sb)
            rt = pool_r.tile([128, TCH], f32, tag="r", name=f"r_{name}")
            nc.vector.reciprocal_approx_fast(out=rt, in_=sd)
            return rt

        # ================= STEM: load x, transpose, LN0+W0 =================
        with (
            tc.tile_pool(name="xtm", bufs=5) as pool_xtm,
            tc.tile_pool(name="xt", bufs=8) as pool_xt,
        ):
            w0c = [None] * 8
            for T in range(NT):
                xtms = []
                for tt in range(4):
                    xtm = pool_xtm.tile(
                        [128, FS], f32r, tag="xtm", name=f"xtm_{T}_{tt}"
                    )
                    r0 = (T * 4 + tt) * 128
                    nc.sync.dma_start(out=xtm, in_=x_d[r0 : r0 + 128, :])
                    xtms.append(xtm)
                xts = []
                for k4 in range(4):
                    xt = pool_xt.tile([128, TCH], f32r, tag="xt", name=f"xt_{T}_{k4}")
                    xts.append(xt)
                for tt in range(4):
                    for k4 in range(4):
                        tps = pool_ps.tile(
                            [128, 128], f32r, tag="z", name=f"tps_{T}_{tt}_{k4}"
                        )
                        nc.tensor.transpose(
                            tps, xtms[tt][:, k4 * 128 : (k4 + 1) * 128], ident
                        )
                        dst = xts[k4][:, tt * 128 : (tt + 1) * 128]
                        if (tt + k4) % 2 == 0:
                            nc.vector.tensor_copy(dst, tps)
                        else:
                            nc.scalar.copy(dst, tps)
                # stats over FS=512
                sqs = []
                for k4 in range(4):
                    sq = pool_sq.tile([128, TCH], f32r, tag="sq", name=f"sqx_{T}_{k4}")
                    nc.scalar.activation(sq, xts[k4], AF.Square)
                    sqs.append(sq)
                S1 = pool_ps.tile([128, TCH], f32, tag="stat", name=f"s1x_{T}")
                for k4 in range(4):
                    nc.tensor.matmul(
                        S1, inv512, xts[k4], start=(k4 == 0), stop=(k4 == 3)
                    )
                S2 = pool_ps.tile([128, TCH], f32, tag="stat", name=f"s2x_{T}")
                for k4 in range(4):
                    nc.tensor.matmul(
                        S2, inv512, sqs[k4], start=(k4 == 0), stop=(k4 == 3)
                    )
                rT = stat_r(S1, S2, T, f"x{T}")
                for m in range(8):
                    if T == 0:
                        w0c[m] = pool_w.tile(
                            [128, 4, 128], f32r, tag="w", name=f"w0c_{m}"
                        )
                        nc.sync.dma_start(out=w0c[m], in_=w0_d[m])
                    zps = pool_ps.tile([128, TCH], f32, tag="z", name=f"z0_{T}_{m}")
                    for k4 in range(4):
                        nc.tensor.matmul(
                            zps,
                            w0c[m][:, k4, :],
                            xts[k4],
                            start=(k4 == 0),
                            stop=(k4 == 3),
                        )
                    u = pool_u.tile([128, TCH], f32, tag="u", name=f"u0_{T}_{m}")
                    nc.vector.tensor_mul(u, zps, rT)
                    h = pool_h.tile([128, TCH], f32r, tag="h", name=f"h_{m}_{T}")
                    nc.scalar.activation(h, u, AF.Relu, bias=c0_sb[:, m : m + 1])
                    h_tiles[m][T] = h

        with ExitStack() as ctx2:
            pool_v = ctx2.enter_context(tc.tile_pool(name="vv", bufs=9))
            pool_fc = ctx2.enter_context(tc.tile_pool(name="fc", bufs=4))
            pool_at = ctx2.enter_context(tc.tile_pool(name="at", bufs=2))

            # ================= RESIDUAL LAYERS =================
            for i in range(L):
                # ---- stats phase (uses h from previous layer)
                for T in range(NT):
                    sqs = []
                    for k in range(8):
                        sq = pool_sq.tile(
                            [128, TCH], f32r, tag="sq", name=f"sqh_{i}_{T}_{k}"
                        )
                        nc.scalar.activation(sq, h_tiles[k][T], AF.Square)
                        sqs.append(sq)
                    S1 = pool_ps.tile([128, TCH], f32, tag="stat", name=f"s1_{i}_{T}")
                    for k in range(8):
                        nc.tensor.matmul(
                            S1, inv1024, h_tiles[k][T], start=(k == 0), stop=(k == 7)
                        )
                    S2 = pool_ps.tile([128, TCH], f32, tag="stat", name=f"s2_{i}_{T}")
                    for k in range(8):
                        nc.tensor.matmul(
                            S2, inv1024, sqs[k], start=(k == 0), stop=(k == 7)
                        )
                    r_tiles[T] = stat_r(S1, S2, T, f"l{i}t{T}")
                # ---- z phase
                wcols = [None] * 8
                for T in range(NT):
                    v2s = []
                    for m in range(8):
                        if T == 0:
                            wcols[m] = pool_w.tile(
                                [128, 8, 128], f32r, tag="w", name=f"w_{i}_{m}"
                            )
                            nc.sync.dma_start(out=wcols[m], in_=wres_d[i, m])
                        zps = pool_ps.tile(
                            [128, TCH], f32, tag="z", name=f"z_{i}_{T}_{m}"
                        )
                        for k in range(8):
                            nc.tensor.matmul(
                                zps,
                                wcols[m][:, k, :],
                                h_tiles[k][T],
                                start=(k == 0),
                                stop=(k == 7),
                            )
                        u = pool_u.tile([128, TCH], f32, tag="u", name=f"u_{i}_{T}_{m}")
                        nc.vector.tensor_mul(u, zps, r_tiles[T])
                        v2 = pool_v.tile(
                            [128, TCH], f32r, tag="v", name=f"v_{i}_{T}_{m}"
                        )
                        nc.scalar.activation(
                            v2, u, AF.Relu, bias=cres_sb[:, i, m : m + 1]
                        )
                        v2s.append(v2)
                    for m in range(8):
                        nc.vector.tensor_add(h_tiles[m][T], h_tiles[m][T], v2s[m])

            # ================= HEAD (lnf + Wf) =================
            fc_tiles = [None] * NT
            for T in range(NT):
                sqs = []
                for k in range(8):
                    sq = pool_sq.tile([128, TCH], f32r, tag="sq", name=f"sqf_{T}_{k}")
                    nc.scalar.activation(sq, h_tiles[k][T], AF.Square)
                    sqs.append(sq)
                S1 = pool_ps.tile([128, TCH], f32, tag="stat", name=f"s1f_{T}")
                for k in range(8):
                    nc.tensor.matmul(
                        S1, inv1024, h_tiles[k][T], start=(k == 0), stop=(k == 7)
                    )
                S2 = pool_ps.tile([128, TCH], f32, tag="stat", name=f"s2f_{T}")
                for k in range(8):
                    nc.tensor.matmul(S2, inv1024, sqs[k], start=(k == 0), stop=(k == 7))
                rT = stat_r(S1, S2, T, f"f{T}")
                if T == 0:
                    wfc = pool_w.tile([128, 8, 128], f32r, tag="w", name="wfc")
                    nc.sync.dma_start(out=wfc, in_=wf_d[:])
                zps = pool_ps.tile([128, TCH], f32, tag="z", name=f"zf_{T}")
                for k in range(8):
                    nc.tensor.matmul(
                        zps, wfc[:, k, :], h_tiles[k][T], start=(k == 0), stop=(k == 7)
                    )
                u = pool_u.tile([128, TCH], f32, tag="u", name=f"uf_{T}")
                nc.vector.tensor_mul(u, zps, rT)
                fcT = pool_fc.tile([128, TCH], f32r, tag="fc", name=f"fc_{T}")
                nc.scalar.activation(fcT, u, AF.Identity, bias=cf_sb[:, 0:1])
                fc_tiles[T] = fcT

            # ================= ATTENTION =================
            wa1c = [None, None]
            wa2c = [None, None]
            for mm in range(2):
                wa1c[mm] = pool_w.tile([128, 2, 128], f32r, tag="w", name=f"wa1c_{mm}")
                nc.sync.dma_start(out=wa1c[mm], in_=wa1_d[mm])
                wa2c[mm] = pool_w.tile([128, 2, 128], f32r, tag="w", name=f"wa2c_{mm}")
                nc.sync.dma_start(out=wa2c[mm], in_=wa2_d[mm])
            wa1l = pool_at.tile([1, 256], f32r, tag="wa1l", bufs=1, name="wa1l")
            nc.sync.dma_start(out=wa1l, in_=wa1l_d[:])

            tT = [
                pool_at.tile([128, BPC, 128], f32r, tag="tT", bufs=2, name=f"tT_{lk}")
                for lk in range(2)
            ]
            for b in range(BPC):
                Tb, coff = b // 2, (b % 2) * 256
                for lk in range(2):
                    tps = pool_ps.tile([128, 128], f32r, tag="z", name=f"tp_{b}_{lk}")
                    nc.tensor.transpose(
                        tps,
                        fc_tiles[Tb][:, coff + lk * 128 : coff + (lk + 1) * 128],
                        ident,
                    )
                    if (b + lk) % 2 == 0:
                        nc.vector.tensor_copy(tT[lk][:, b, :], tps)
                    else:
                        nc.scalar.copy(tT[lk][:, b, :], tps)
            tmean = pool_at.tile([1, BPC, 128], f32r, tag="tmean", bufs=1, name="tmean")
            for n in range(2):
                nsl = slice(n * 4, (n + 1) * 4)
                mps = pool_ps.tile([1, TCH], f32, tag="stat", name=f"mps_{n}")
                for lk in range(2):
                    nc.tensor.matmul(
                        mps,
                        inv256[:, 0:1],
                        tT[lk][:, nsl, :].rearrange("p b j -> p (b j)"),
                        start=(lk == 0),
                        stop=(lk == 1),
                    )
                nc.vector.tensor_copy(
                    tmean[:, nsl, :].rearrange("p b j -> p (b j)"), mps
                )

            for n in range(2):
                nsl = slice(n * 4, (n + 1) * 4)
                t0n = tT[0][:, nsl, :].rearrange("p b j -> p (b j)")
                t1n = tT[1][:, nsl, :].rearrange("p b j -> p (b j)")
                tmn = tmean[:, nsl, :].rearrange("p b j -> p (b j)")
                # ---- t stats (F=257)
                sq0 = pool_sq.tile([128, TCH], f32r, tag="sq", name=f"sqt0_{n}")
                nc.scalar.activation(sq0, t0n, AF.Square)
                sq1 = pool_sq.tile([128, TCH], f32r, tag="sq", name=f"sqt1_{n}")
                nc.scalar.activation(sq1, t1n, AF.Square)
                sqm = pool_sq.tile([1, TCH], f32r, tag="sqm", bufs=2, name=f"sqm_{n}")
                nc.scalar.activation(sqm, tmn, AF.Square)
                S1 = pool_ps.tile([128, TCH], f32, tag="stat", name=f"s1t_{n}")
                nc.tensor.matmul(S1, inv257, t0n, start=True, stop=False)
                nc.tensor.matmul(S1, inv257, t1n, start=False, stop=False)
                nc.tensor.matmul(S1, inv257[0:1, :], tmn, start=False, stop=True)
                S2 = pool_ps.tile([128, TCH], f32, tag="stat", name=f"s2t_{n}")
                nc.tensor.matmul(S2, inv257, sq0, start=True, stop=False)
                nc.tensor.matmul(S2, inv257, sq1, start=False, stop=False)
                nc.tensor.matmul(S2, inv257[0:1, :], sqm, start=False, stop=True)
                r1n = stat_r(S1, S2, 0, f"a1_{n}")
                # ---- a = relu(r1*(t@Wa1)+ca1)
                a_sb = []
                for ma in range(2):
                    aps = pool_ps.tile([128, TCH], f32, tag="z", name=f"aps_{n}_{ma}")
                    nc.tensor.matmul(aps, wa1c[ma][:, 0, :], t0n, start=True, stop=False)
                    nc.tensor.matmul(aps, wa1c[ma][:, 1, :], t1n, start=False, stop=False)
                    nc.tensor.matmul(
                        aps,
                        wa1l[:, ma * 128 : (ma + 1) * 128],
                        tmn,
                        start=False,
                        stop=True,
                    )
                    u = pool_u.tile([128, TCH], f32, tag="u", name=f"ua_{n}_{ma}")
                    nc.vector.tensor_mul(u, aps, r1n)
                    av = pool_at.tile(
                        [128, TCH], f32r, tag="a", bufs=2, name=f"a_{n}_{ma}"
                    )
                    nc.scalar.activation(av, u, AF.Relu, bias=ca1_sb[:, ma : ma + 1])
                    a_sb.append(av)
                # ---- a stats (F=256)
                sqa = []
                for ka in range(2):
                    sq = pool_sq.tile([128, TCH], f32r, tag="sq", name=f"sqa_{n}_{ka}")
                    nc.scalar.activation(sq, a_sb[ka], AF.Square)
                    sqa.append(sq)
                S1a = pool_ps.tile([128, TCH], f32, tag="stat", name=f"s1a_{n}")
                for ka in range(2):
                    nc.tensor.matmul(
                        S1a, inv256, a_sb[ka], start=(ka == 0), stop=(ka == 1)
                    )
                S2a = pool_ps.tile([128, TCH], f32, tag="stat", name=f"s2a_{n}")
                for ka in range(2):
                    nc.tensor.matmul(
                        S2a, inv256, sqa[ka], start=(ka == 0), stop=(ka == 1)
                    )
                r2n = stat_r(S1a, S2a, 0, f"a2_{n}")
                # ---- w logits + exp
                e_sb = []
                for ml in range(2):
                    wps = pool_ps.tile([128, TCH], f32, tag="z", name=f"wps_{n}_{ml}")
                    for ka in range(2):
                        nc.tensor.matmul(
                            wps,
                            wa2c[ml][:, ka, :],
                            a_sb[ka],
                            start=(ka == 0),
                            stop=(ka == 1),
                        )
                    u = pool_u.tile([128, TCH], f32, tag="u", name=f"uw_{n}_{ml}")
                    nc.vector.tensor_mul(u, wps, r2n)
                    ev = pool_at.tile(
                        [128, TCH], f32r, tag="e", bufs=2, name=f"e_{n}_{ml}"
                    )
                    nc.scalar.activation(ev, u, AF.Exp, bias=ca2_sb[:, ml : ml + 1])
                    e_sb.append(ev)
                esum = pool_ps.tile([1, TCH], f32, tag="stat", name=f"esum_{n}")
                for ml in range(2):
                    nc.tensor.matmul(
                        esum, ones_c[:, 0:1], e_sb[ml], start=(ml == 0), stop=(ml == 1)
                    )
                p_sb = []
                for ml in range(2):
                    pv = pool_at.tile(
                        [128, TCH], f32r, tag="p", bufs=2, name=f"p_{n}_{ml}"
                    )
                    tml = tT[ml][:, nsl, :].rearrange("p b j -> p (b j)")
                    nc.vector.tensor_mul(pv, tml, e_sb[ml])
                    p_sb.append(pv)
                xe = pool_ps.tile([1, TCH], f32, tag="stat", name=f"xe_{n}")
                for ml in range(2):
                    nc.tensor.matmul(
                        xe, ones_c[:, 0:1], p_sb[ml], start=(ml == 0), stop=(ml == 1)
                    )
                den = pool_at.tile([1, TCH], f32, tag="fin", bufs=4, name=f"den_{n}")
                nc.vector.tensor_copy(den, esum)
                num = pool_at.tile([1, TCH], f32, tag="fin", bufs=4, name=f"num_{n}")
                nc.scalar.copy(num, xe)
                rden = pool_at.tile([1, TCH], f32, tag="fin", bufs=4, name=f"rden_{n}")
                nc.vector.reciprocal_approx_fast(out=rden, in_=den)
                outv = pool_at.tile([1, TCH], f32, tag="fin", bufs=4, name=f"outv_{n}")
                nc.vector.tensor_mul(outv, num, rden)
                nc.sync.dma_start(out=out_d[:, n * TCH : (n + 1) * TCH], in_=outv)


# ---------------------------------------------------------------- jax runner
def _get_runner():
    if "runner" in _CACHE:
        return _CACHE["runner"]

    import jax
    from jax.sharding import Mesh, PartitionSpec
    try:
        from jax.experimental.shard_map import shard_map
    except ImportError:
        from jax.shard_map import shard_map

    import concourse.mybir as mybir
    from concourse import bass2jax
    from concourse.bass2jax import (
        _bass_exec_p,
        install_neuronx_cc_hook,
        partition_id_tensor,
    )

    nc = _build()
    install_neuronx_cc_hook()

    partition_name = nc.partition_id_tensor.name if nc.partition_id_tensor else None
    in_names = []
    out_names = []
    out_avals = []
    zero_outs = []
    for alloc in nc.m.functions[0].allocations:
        if not isinstance(alloc, mybir.MemoryLocationSet):
            continue
        name = alloc.memorylocations[0].name
        if alloc.kind == "ExternalInput":
            if name == partition_name:
                continue
            in_names.append(name)
        elif alloc.kind == "ExternalOutput":
            out_names.append(name)
            shape = tuple(alloc.tensor_shape)
            dtype = mybir.dt.np(alloc.dtype)
            out_avals.append(jax.core.ShapedArray(shape, dtype))
            zero_outs.append(np.zeros(shape, dtype))
    n_params = len(in_names)
    n_outs = len(out_avals)
    all_names = in_names + out_names
    if partition_name is not None:
        all_names = all_names + [partition_name]
    donate = tuple(range(n_params, n_params + n_outs))

    def _body(*args):
        operands = list(args)
        if partition_name is not None:
            operands.append(partition_id_tensor())
        outs = _bass_exec_p.bind(
            *operands,
            out_avals=tuple(out_avals),
            in_names=tuple(all_names),
            out_names=tuple(out_names),
            lowering_input_output_aliases=(),
            sim_require_finite=True,
            sim_require_nnan=True,
            nc=nc,
        )
        return tuple(outs)

    devices = jax.devices()[:NCORES]
    mesh = Mesh(np.asarray(devices), ("core",))
    in_specs = (PartitionSpec("core"),) * (n_params + n_outs)
    out_specs = (PartitionSpec("core"),) * n_outs
    sharded = jax.jit(
        shard_map(
            _body, mesh=mesh, in_specs=in_specs, out_specs=out_specs, check_rep=False
        ),
        donate_argnums=donate,
        keep_unused=True,
    )

    runner = {
        "nc": nc,
        "sharded": sharded,
        "in_names": in_names,
        "out_names": out_names,
        "out_avals": out_avals,
        "zero_outs": zero_outs,
        "mesh": mesh,
        "jax": jax,
    }
    _CACHE["runner"] = runner
    return runner


def _concat_inputs(maps, runner):
    return [
        np.concatenate([maps[c][name] for c in range(NCORES)], axis=0)
        for name in runner["in_names"]
    ]


def _run(maps):
    r = _get_runner()
    concat_in = _concat_inputs(maps, r)
    concat_zeros = [
        np.zeros((NCORES * z.shape[0], *z.shape[1:]), z.dtype) for z in r["zero_outs"]
    ]
    out_arrs = r["sharded"](*concat_in, *concat_zeros)
    outs = []
    for c in range(NCORES):
        outs.append(
            {
                name: np.asarray(out_arrs[i]).reshape(
                    NCORES, *r["out_avals"][i].shape
                )[c]
                for i, name in enumerate(r["out_names"])
            }
        )
    return outs


def kernel(**inputs):
    maps = _prepare(inputs)
    results = _run(maps)
    out = np.concatenate(
        [results[c]["out"].reshape(BPC, OUT) for c in range(NCORES)], axis=0
    )
    return np.ascontiguousarray(out, dtype=np.float32)


# Timing helper for test.py: device-resident inputs, repeated execution.
def timed_run(inputs, iters=20):
    import jax

    maps = _prepare(inputs)
    r = _get_runner()
    concat_in = _concat_inputs(maps, r)
    dev_in = [jax.device_put(a) for a in concat_in]
    times = []
    out_arrs = None
    for _ in range(iters):
        concat_zeros = [
            jax.device_put(np.zeros((NCORES * z.shape[0], *z.shape[1:]), z.dtype))
            for z in r["zero_outs"]
        ]
        jax.block_until_ready(concat_zeros)
        t0 = time.perf_counter()
        out_arrs = r["sharded"](*dev_in, *concat_zeros)
        jax.block_until_ready(out_arrs)
        times.append(time.perf_counter() - t0)
    out = np.concatenate(
        [
            np.asarray(out_arrs[0]).reshape(NCORES, BPC, OUT)[c]
            for c in range(NCORES)
        ],
        axis=0,
    )
    return np.ascontiguousarray(out, np.float32), times
